# revision 15
# baseline (speedup 1.0000x reference)
"""Trainium2 Bass kernel for a dense causal-attention transformer block.

Computes: qkv projections + RoPE + causal softmax attention + output
projection, matching the reference jax implementation with
B=2, S=2048, D=2048, 16 heads x 128 head-dim, on 8 NeuronCores.

Sharding: data-parallel over batch (2 groups of 4 cores) x tensor-parallel
over heads (4 heads per core). Attention is fully head-local; the only
communication is an AllGather of the per-core attention outputs (bf16)
within each 4-core batch group before the wo matmul, of which each core
computes a 512-wide output-column slice.
"""
import os
import sys
import types

sys.path.insert(0, "/opt/trn_rl_repo")

import numpy as np


def _install_ntff_hook():
    """Recreate the missing antenv.axon_hooks module so trace=True works."""
    try:
        import antenv

        if "antenv.axon_hooks" in sys.modules:
            return
        m = types.ModuleType("antenv.axon_hooks")
        m._hook = None

        def set_axon_ntff_profile_hook(h):
            m._hook = h

        def get_axon_ntff_profile_hook():
            return m._hook

        m.set_axon_ntff_profile_hook = set_axon_ntff_profile_hook
        m.get_axon_ntff_profile_hook = get_axon_ntff_profile_hook
        sys.modules["antenv.axon_hooks"] = m
        antenv.axon_hooks = m
        from trn_agent_boot.trn_boot import _ntff_profile_via_ctypes

        so = "/opt/axon/libaxon_pjrt.so"
        if os.path.exists(so):
            set_axon_ntff_profile_hook(_ntff_profile_via_ctypes(so))
    except Exception:
        pass


_install_ntff_hook()

import ml_dtypes
import concourse.bass as bass
import concourse.tile as tile
from concourse import bacc, mybir
from concourse.bass_utils import run_bass_kernel_spmd

BF16 = mybir.dt.bfloat16
F32 = mybir.dt.float32

B, S, D = 2, 2048, 2048
QH, H = 16, 128          # heads, head dim
N_CORES = 8
GROUPS = 4               # tensor-parallel groups per batch
HPC = QH // GROUPS       # heads per core = 4
DQC = HPC * H            # per-core projection width = 512
NT = S // 128            # 16 s/d tiles of 128
NC = S // 512            # 4 chunks of 512
KT = D // 128            # 16 contraction tiles

LAST_RESULTS = None      # test harness reads exec_time_ns from here


def _build():
    nc = bacc.Bacc("TRN2", target_bir_lowering=False, debug=False,
                   num_devices=N_CORES)

    xt = nc.dram_tensor("xt", [D, S], BF16, kind="ExternalInput")
    wq = nc.dram_tensor("wq", [D, DQC], BF16, kind="ExternalInput")
    wk = nc.dram_tensor("wk", [D, DQC], BF16, kind="ExternalInput")
    wv = nc.dram_tensor("wv", [D, DQC], BF16, kind="ExternalInput")
    wo = nc.dram_tensor("wo", [D, DQC], BF16, kind="ExternalInput")
    cosT = nc.dram_tensor("cosT", [H, S], BF16, kind="ExternalInput")
    sinTs = nc.dram_tensor("sinTs", [H, S], BF16, kind="ExternalInput")
    tri = nc.dram_tensor("tri", [128, 128], BF16, kind="ExternalInput")
    out = nc.dram_tensor("out", [S, DQC], F32, kind="ExternalOutput")

    with tile.TileContext(nc) as tc:
        with (
            tc.tile_pool(name="big", bufs=16) as big_pool,        # xt / ytf tiles
            tc.tile_pool(name="wts", bufs=16) as wts_pool,        # weight tiles
            tc.tile_pool(name="qkv", bufs=1) as qkv_pool,         # qt/kt/v/yt
            tc.tile_pool(name="small", bufs=1) as small_pool,     # constants
            tc.tile_pool(name="work", bufs=3) as work_pool,       # rope/at/tmp
            tc.tile_pool(name="psum", bufs=2, space="PSUM") as psum_pool,
            tc.tile_pool(name="dram", bufs=1, space="DRAM") as dram_pool,
        ):
            # ---- constants / inputs to SBUF -------------------------------
            cos_sb = small_pool.tile([H, S], BF16, tag="cos")
            nc.sync.dma_start(cos_sb[:], cosT[:])
            sin_sb = small_pool.tile([H, S], BF16, tag="sin")
            nc.sync.dma_start(sin_sb[:], sinTs[:])
            tri_sb = small_pool.tile([128, 128], BF16, tag="tri")
            nc.sync.dma_start(tri_sb[:], tri[:])
            ones_sb = small_pool.tile([128, 1], BF16, tag="ones")
            nc.vector.memset(ones_sb[:], 1.0)

            xt_sb = []
            wq_sb, wk_sb = [], []
            for kd in range(KT):
                r = slice(kd * 128, (kd + 1) * 128)
                t = big_pool.tile([128, S], BF16, tag="big", name=f"xt{kd}")
                nc.sync.dma_start(t[:], xt[r, :])
                xt_sb.append(t)
                for (lst, src, nm, tg) in ((wq_sb, wq, "wq", "w0"),
                                           (wk_sb, wk, "wk", "w1")):
                    w = wts_pool.tile([128, DQC], BF16, tag=tg,
                                      name=f"{nm}{kd}")
                    nc.sync.dma_start(w[:], src[r, :])
                    lst.append(w)

            # ---- pipelined stages ----------------------------------------
            # Per head-block m: project Q/K (head 0 first), attention, then
            # fire that head's AllGather piece while later heads project and
            # attend; stage F consumes gathered pieces as they arrive.
            qt_sb = [qkv_pool.tile([H, S], BF16, tag=f"qt{m}", name=f"qt{m}")
                     for m in range(HPC)]
            kt_sb = [qkv_pool.tile([H, S], BF16, tag=f"kt{m}", name=f"kt{m}")
                     for m in range(HPC)]
            v_sb = [qkv_pool.tile([128, DQC], BF16, tag=f"v{i}", name=f"v{i}")
                    for i in range(NT)]

            def proj_head(m):
                """Q^T and K^T projections + RoPE for head m."""
                for sc in range(NC):
                    scol = slice(sc * 512, (sc + 1) * 512)
                    for (w_list, dst) in ((wq_sb, qt_sb[m]),
                                          (wk_sb, kt_sb[m])):
                        ps = psum_pool.tile([128, 512], F32, tag="ps_big",
                                            bufs=2, name="ps")
                        for kd in range(KT):
                            nc.tensor.matmul(
                                ps[:],
                                w_list[kd][:, m * 128:(m + 1) * 128],
                                xt_sb[kd][:, scol],
                                start=(kd == 0), stop=(kd == KT - 1),
                            )
                        # RoPE: out = p*cos + swap_halves(p)*sin_signed
                        t_rot = work_pool.tile([128, 512], F32, tag="t_rot",
                                               bufs=2, name="t_rot")
                        nc.vector.tensor_copy(t_rot[0:64, :], ps[64:128, :])
                        nc.vector.tensor_copy(t_rot[64:128, :], ps[0:64, :])
                        t_cos = work_pool.tile([128, 512], F32, tag="t_cos",
                                               bufs=2, name="t_cos")
                        nc.vector.tensor_tensor(
                            t_cos[:], ps[:], cos_sb[:, scol],
                            mybir.AluOpType.mult)
                        t_sin = work_pool.tile([128, 512], F32, tag="t_sin",
                                               bufs=2, name="t_sin")
                        nc.vector.tensor_tensor(
                            t_sin[:], t_rot[:], sin_sb[:, scol],
                            mybir.AluOpType.mult)
                        nc.vector.tensor_tensor(
                            dst[:, scol], t_cos[:], t_sin[:],
                            mybir.AluOpType.add)

            for m in range(HPC):
                proj_head(m)

            # wv gets its own slots (wq/wk stay live for later heads);
            # wo reuses them after the V projection drains
            wv_sb = []
            for kd in range(KT):
                w = wts_pool.tile([128, DQC], BF16, tag="w2", name=f"wv{kd}")
                nc.sync.dma_start(w[:], wv[kd * 128:(kd + 1) * 128, :])
                wv_sb.append(w)

            for i in range(NT):
                ps = psum_pool.tile([128, DQC], F32, tag="ps_big", bufs=2,
                                    name="ps_v")
                for kd in range(KT):
                    nc.tensor.matmul(
                        ps[:],
                        xt_sb[kd][:, i * 128:(i + 1) * 128],
                        wv_sb[kd][:],
                        start=(kd == 0), stop=(kd == KT - 1),
                    )
                nc.scalar.copy(v_sb[i][:], ps[:])

            wo_sb = []
            for kd in range(KT):
                w = wts_pool.tile([128, DQC], BF16, tag="w2", name=f"wo{kd}")
                nc.sync.dma_start(w[:], wo[kd * 128:(kd + 1) * 128, :])
                wo_sb.append(w)

            yt_piece = [dram_pool.tile([128, S], BF16, name=f"yt_p{m}")
                        for m in range(HPC)]
            ytf_piece = [dram_pool.tile([DQC, S], BF16, name=f"ytf_p{m}")
                         for m in range(HPC)]

            def attn_head(m):
                """Causal attention for head m -> yt_piece[m]."""
                for j in range(NC):
                    n_kv = 4 * j + 4
                    ps_y = psum_pool.tile([128, 512], F32, tag="ps_y",
                                          name="ps_y")
                    ps_d = psum_pool.tile([1, 512], F32, tag="ps_d",
                                          name="ps_d")
                    for t in range(n_kv):
                        c0 = max(0, (t - 4 * j) * 128)
                        ps_s = psum_pool.tile([128, 512], F32, tag="ps_s",
                                              name="ps_s")
                        nc.tensor.matmul(
                            ps_s[:, c0:512],
                            kt_sb[m][:, t * 128:(t + 1) * 128],
                            qt_sb[m][:, j * 512 + c0:(j + 1) * 512],
                            start=True, stop=True,
                        )
                        at = work_pool.tile([128, 512], BF16, tag="at",
                                            bufs=3, name="at")
                        nc.scalar.activation(
                            at[:, c0:512], ps_s[:, c0:512],
                            mybir.ActivationFunctionType.Exp)
                        if t >= 4 * j:
                            nc.vector.tensor_tensor(
                                at[:, c0:c0 + 128], at[:, c0:c0 + 128],
                                tri_sb[:], mybir.AluOpType.mult)
                        nc.tensor.matmul(
                            ps_y[:, c0:512],
                            v_sb[t][:, m * 128:(m + 1) * 128],
                            at[:, c0:512],
                            start=(t == 0), stop=(t == n_kv - 1),
                        )
                        nc.tensor.matmul(
                            ps_d[:, c0:512],
                            ones_sb[:],
                            at[:, c0:512],
                            start=(t == 0), stop=(t == n_kv - 1),
                        )
                    d_sb = work_pool.tile([1, 512], F32, tag="d_sb", bufs=2,
                                          name="d_sb")
                    nc.vector.reciprocal(d_sb[:], ps_d[:])
                    b_sb = work_pool.tile([128, 512], F32, tag="b_sb", bufs=2,
                                          name="b_sb")
                    nc.gpsimd.partition_broadcast(b_sb[:], d_sb[:])
                    ytile = work_pool.tile([128, 512], BF16, tag="ytile",
                                           bufs=2, name="ytile")
                    nc.vector.tensor_tensor(
                        ytile[:], ps_y[:], b_sb[:], mybir.AluOpType.mult)
                    nc.sync.dma_start(
                        yt_piece[m][:, j * 512:(j + 1) * 512], ytile[:])

            # attention train: per head attention -> AllGather piece ->
            # immediate SBUF loads of the gathered piece (x tiles are dead
            # once the projections above finish, so the big slots are free)
            ytfp_sb = [[None] * GROUPS for _ in range(HPC)]
            for m in range(HPC):
                attn_head(m)
                nc.gpsimd.collective_compute(
                    "AllGather",
                    mybir.AluOpType.bypass,
                    replica_groups=[[0, 1, 2, 3], [4, 5, 6, 7]],
                    ins=[yt_piece[m].opt()],
                    outs=[ytf_piece[m].opt()],
                )
                for p in range(GROUPS):
                    t = big_pool.tile([128, S], BF16, tag="big",
                                      name=f"ytf{m}_{p}")
                    nc.sync.dma_start(
                        t[:], ytf_piece[m][p * 128:(p + 1) * 128, :])
                    ytfp_sb[m][p] = t

            # ---- stage F: output projection (512-col slice) ---------------
            # ytf_piece[m] rows: 4 ranks x head m -> global head 4*p + m

            for ms in range(NT):
                ps = psum_pool.tile([128, DQC], F32, tag="ps_big", bufs=2,
                                    name="ps_o")
                for m in range(HPC):
                    for p in range(GROUPS):
                        nc.tensor.matmul(
                            ps[:],
                            ytfp_sb[m][p][:, ms * 128:(ms + 1) * 128],
                            wo_sb[4 * p + m][:],
                            start=(m == 0 and p == 0),
                            stop=(m == HPC - 1 and p == GROUPS - 1),
                        )
                o_sb = work_pool.tile([128, DQC], F32, tag="o_sb", bufs=2,
                                      name="o_sb")
                nc.scalar.copy(o_sb[:], ps[:])
                nc.sync.dma_start(out[ms * 128:(ms + 1) * 128, :], o_sb[:])

    nc.compile()
    return nc


_NC_CACHE = None


def kernel(x, wq, wk, wv, wo, mask, sin, cos):
    global LAST_RESULTS, _NC_CACHE
    bf16 = ml_dtypes.bfloat16

    xt = np.ascontiguousarray(x.transpose(0, 2, 1)).astype(bf16)     # [B, D, S]
    wq_b = wq.astype(bf16)
    wk_b = (wk * (H ** -0.5)).astype(bf16)   # fold k scaling into wk
    wv_b = wv.astype(bf16)
    wo_b = wo.astype(bf16)

    # transposed rope tables; sign-folded sin handles rotate_half:
    #   q'[0:64]   = q[0:64]*cos[0:64]   + q[64:128]*(-sin[0:64])
    #   q'[64:128] = q[64:128]*cos[64:]  + q[0:64]  *(+sin[64:])
    cosT = np.ascontiguousarray(cos.T).astype(bf16)                  # [H, S]
    sinT = np.ascontiguousarray(sin.T).astype(np.float32)
    sinTs = sinT.copy()
    sinTs[0:H // 2, :] *= -1.0
    sinTs = sinTs.astype(bf16)

    # multiplicative causal mask for the 128x128 diagonal blocks, in
    # [kv, q] orientation, derived from the additive mask input
    tri = (mask[:128, :128].T == 0.0).astype(bf16)

    if _NC_CACHE is None:
        _NC_CACHE = _build()
    nc = _NC_CACHE

    in_maps = []
    for c in range(N_CORES):
        b, g = c // GROUPS, c % GROUPS
        cols = slice(g * DQC, (g + 1) * DQC)
        in_maps.append({
            "xt": xt[b],
            "wq": np.ascontiguousarray(wq_b[:, cols]),
            "wk": np.ascontiguousarray(wk_b[:, cols]),
            "wv": np.ascontiguousarray(wv_b[:, cols]),
            "wo": np.ascontiguousarray(wo_b[:, cols]),
            "cosT": cosT,
            "sinTs": sinTs,
            "tri": tri,
        })

    res = run_bass_kernel_spmd(nc, in_maps, core_ids=list(range(N_CORES)))
    LAST_RESULTS = res

    output = np.empty((B, S, D), dtype=np.float32)
    for c in range(N_CORES):
        b, g = c // GROUPS, c % GROUPS
        output[b, :, g * DQC:(g + 1) * DQC] = res.results[c]["out"]
    return output


# revision 16
# speedup vs baseline: 1.0993x; 1.0993x over previous
"""Trainium2 Bass kernel for a dense causal-attention transformer block.

Computes: qkv projections + RoPE + causal softmax attention + output
projection, matching the reference jax implementation with
B=2, S=2048, D=2048, 16 heads x 128 head-dim, on 8 NeuronCores.

Sharding: data-parallel over batch (2 groups of 4 cores) x tensor-parallel
over heads (4 heads per core). Attention is fully head-local; the only
communication is an AllGather of the per-core attention outputs (bf16)
within each 4-core batch group before the wo matmul, of which each core
computes a 512-wide output-column slice.
"""
import os
import sys
import types

sys.path.insert(0, "/opt/trn_rl_repo")

import numpy as np


def _install_ntff_hook():
    """Recreate the missing antenv.axon_hooks module so trace=True works."""
    try:
        import antenv

        if "antenv.axon_hooks" in sys.modules:
            return
        m = types.ModuleType("antenv.axon_hooks")
        m._hook = None

        def set_axon_ntff_profile_hook(h):
            m._hook = h

        def get_axon_ntff_profile_hook():
            return m._hook

        m.set_axon_ntff_profile_hook = set_axon_ntff_profile_hook
        m.get_axon_ntff_profile_hook = get_axon_ntff_profile_hook
        sys.modules["antenv.axon_hooks"] = m
        antenv.axon_hooks = m
        from trn_agent_boot.trn_boot import _ntff_profile_via_ctypes

        so = "/opt/axon/libaxon_pjrt.so"
        if os.path.exists(so):
            set_axon_ntff_profile_hook(_ntff_profile_via_ctypes(so))
    except Exception:
        pass


_install_ntff_hook()

import ml_dtypes
import concourse.bass as bass
import concourse.tile as tile
from concourse import bacc, mybir
from concourse.bass_utils import run_bass_kernel_spmd

BF16 = mybir.dt.bfloat16
F32 = mybir.dt.float32

B, S, D = 2, 2048, 2048
QH, H = 16, 128          # heads, head dim
N_CORES = 8
GROUPS = 4               # tensor-parallel groups per batch
HPC = QH // GROUPS       # heads per core = 4
DQC = HPC * H            # per-core projection width = 512
NT = S // 128            # 16 s/d tiles of 128
NC = S // 512            # 4 chunks of 512
KT = D // 128            # 16 contraction tiles

LAST_RESULTS = None      # test harness reads exec_time_ns from here


def _build():
    nc = bacc.Bacc("TRN2", target_bir_lowering=False, debug=False,
                   num_devices=N_CORES)

    xt = nc.dram_tensor("xt", [D, S], BF16, kind="ExternalInput")
    wq = nc.dram_tensor("wq", [D, DQC], BF16, kind="ExternalInput")
    wk = nc.dram_tensor("wk", [D, DQC], BF16, kind="ExternalInput")
    wv = nc.dram_tensor("wv", [D, DQC], BF16, kind="ExternalInput")
    wo = nc.dram_tensor("wo", [D, DQC], BF16, kind="ExternalInput")
    cosT = nc.dram_tensor("cosT", [H, S], BF16, kind="ExternalInput")
    sinTs = nc.dram_tensor("sinTs", [H, S], BF16, kind="ExternalInput")
    tri = nc.dram_tensor("tri", [128, 128], BF16, kind="ExternalInput")
    out = nc.dram_tensor("out", [S, DQC], F32, kind="ExternalOutput")

    with tile.TileContext(nc) as tc:
        with (
            tc.tile_pool(name="big", bufs=16) as big_pool,        # xt / ytf tiles
            tc.tile_pool(name="wts", bufs=16) as wts_pool,        # weight tiles
            tc.tile_pool(name="qkv", bufs=1) as qkv_pool,         # qt/kt/v/yt
            tc.tile_pool(name="small", bufs=1) as small_pool,     # constants
            tc.tile_pool(name="work", bufs=3) as work_pool,       # rope/at/tmp
            tc.tile_pool(name="psum", bufs=2, space="PSUM") as psum_pool,
            tc.tile_pool(name="dram", bufs=1, space="DRAM") as dram_pool,
        ):
            # ---- constants / inputs to SBUF -------------------------------
            cos_sb = small_pool.tile([H, S], BF16, tag="cos")
            nc.sync.dma_start(cos_sb[:], cosT[:])
            sin_sb = small_pool.tile([H, S], BF16, tag="sin")
            nc.sync.dma_start(sin_sb[:], sinTs[:])
            tri_sb = small_pool.tile([128, 128], BF16, tag="tri")
            nc.sync.dma_start(tri_sb[:], tri[:])
            ones_sb = small_pool.tile([128, 1], BF16, tag="ones")
            nc.vector.memset(ones_sb[:], 1.0)

            xt_sb = []
            wq_sb, wk_sb = [], []
            for kd in range(KT):
                r = slice(kd * 128, (kd + 1) * 128)
                t = big_pool.tile([128, S], BF16, tag="big", name=f"xt{kd}")
                nc.sync.dma_start(t[:], xt[r, :])
                xt_sb.append(t)
                for (lst, src, nm, tg) in ((wq_sb, wq, "wq", "w0"),
                                           (wk_sb, wk, "wk", "w1")):
                    w = wts_pool.tile([128, DQC], BF16, tag=tg,
                                      name=f"{nm}{kd}")
                    nc.sync.dma_start(w[:], src[r, :])
                    lst.append(w)

            # ---- pipelined stages ----------------------------------------
            # Per head-block m: project Q/K (head 0 first), attention, then
            # fire that head's AllGather piece while later heads project and
            # attend; stage F consumes gathered pieces as they arrive.
            qt_sb = [qkv_pool.tile([H, S], BF16, tag=f"qt{m}", name=f"qt{m}")
                     for m in range(HPC)]
            kt_sb = [qkv_pool.tile([H, S], BF16, tag=f"kt{m}", name=f"kt{m}")
                     for m in range(HPC)]
            v_sb = [qkv_pool.tile([128, DQC], BF16, tag=f"v{i}", name=f"v{i}")
                    for i in range(NT)]

            def proj_head(m):
                """Q^T and K^T projections + RoPE for head m."""
                for sc in range(NC):
                    scol = slice(sc * 512, (sc + 1) * 512)
                    for (w_list, dst) in ((wq_sb, qt_sb[m]),
                                          (wk_sb, kt_sb[m])):
                        ps = psum_pool.tile([128, 512], F32, tag="ps_big",
                                            bufs=2, name="ps")
                        for kd in range(KT):
                            nc.tensor.matmul(
                                ps[:],
                                w_list[kd][:, m * 128:(m + 1) * 128],
                                xt_sb[kd][:, scol],
                                start=(kd == 0), stop=(kd == KT - 1),
                            )
                        # RoPE: out = p*cos + swap_halves(p)*sin_signed
                        t_rot = work_pool.tile([128, 512], F32, tag="t_rot",
                                               bufs=2, name="t_rot")
                        nc.vector.tensor_copy(t_rot[0:64, :], ps[64:128, :])
                        nc.vector.tensor_copy(t_rot[64:128, :], ps[0:64, :])
                        t_cos = work_pool.tile([128, 512], F32, tag="t_cos",
                                               bufs=2, name="t_cos")
                        nc.vector.tensor_tensor(
                            t_cos[:], ps[:], cos_sb[:, scol],
                            mybir.AluOpType.mult)
                        t_sin = work_pool.tile([128, 512], F32, tag="t_sin",
                                               bufs=2, name="t_sin")
                        nc.vector.tensor_tensor(
                            t_sin[:], t_rot[:], sin_sb[:, scol],
                            mybir.AluOpType.mult)
                        nc.vector.tensor_tensor(
                            dst[:, scol], t_cos[:], t_sin[:],
                            mybir.AluOpType.add)

            proj_head(0)

            # wv gets its own slots (wq/wk stay live for later heads);
            # wo reuses them after the V projection drains
            wv_sb = []
            for kd in range(KT):
                w = wts_pool.tile([128, DQC], BF16, tag="w2", name=f"wv{kd}")
                nc.sync.dma_start(w[:], wv[kd * 128:(kd + 1) * 128, :])
                wv_sb.append(w)

            for i in range(NT):
                ps = psum_pool.tile([128, DQC], F32, tag="ps_big", bufs=2,
                                    name="ps_v")
                for kd in range(KT):
                    nc.tensor.matmul(
                        ps[:],
                        xt_sb[kd][:, i * 128:(i + 1) * 128],
                        wv_sb[kd][:],
                        start=(kd == 0), stop=(kd == KT - 1),
                    )
                nc.scalar.copy(v_sb[i][:], ps[:])

            wo_sb = []
            for kd in range(KT):
                w = wts_pool.tile([128, DQC], BF16, tag="w2", name=f"wo{kd}")
                nc.sync.dma_start(w[:], wo[kd * 128:(kd + 1) * 128, :])
                wo_sb.append(w)

            yt_piece = [dram_pool.tile([128, S], BF16, name=f"yt_p{m}")
                        for m in range(HPC)]
            ytf_piece = [dram_pool.tile([DQC, S], BF16, name=f"ytf_p{m}")
                         for m in range(HPC)]

            def attn_head(m):
                """Causal attention for head m -> yt_piece[m]."""
                for j in range(NC):
                    n_kv = 4 * j + 4
                    ps_y = psum_pool.tile([128, 512], F32, tag="ps_y",
                                          name="ps_y")
                    ps_d = psum_pool.tile([1, 512], F32, tag="ps_d",
                                          name="ps_d")
                    for t in range(n_kv):
                        c0 = max(0, (t - 4 * j) * 128)
                        ps_s = psum_pool.tile([128, 512], F32, tag="ps_s",
                                              name="ps_s")
                        nc.tensor.matmul(
                            ps_s[:, c0:512],
                            kt_sb[m][:, t * 128:(t + 1) * 128],
                            qt_sb[m][:, j * 512 + c0:(j + 1) * 512],
                            start=True, stop=True,
                        )
                        at = work_pool.tile([128, 512], BF16, tag="at",
                                            bufs=3, name="at")
                        nc.scalar.activation(
                            at[:, c0:512], ps_s[:, c0:512],
                            mybir.ActivationFunctionType.Exp)
                        if t >= 4 * j:
                            nc.vector.tensor_tensor(
                                at[:, c0:c0 + 128], at[:, c0:c0 + 128],
                                tri_sb[:], mybir.AluOpType.mult)
                        nc.tensor.matmul(
                            ps_y[:, c0:512],
                            v_sb[t][:, m * 128:(m + 1) * 128],
                            at[:, c0:512],
                            start=(t == 0), stop=(t == n_kv - 1),
                        )
                        nc.tensor.matmul(
                            ps_d[:, c0:512],
                            ones_sb[:],
                            at[:, c0:512],
                            start=(t == 0), stop=(t == n_kv - 1),
                        )
                    d_sb = work_pool.tile([1, 512], F32, tag="d_sb", bufs=2,
                                          name="d_sb")
                    nc.vector.reciprocal(d_sb[:], ps_d[:])
                    b_sb = work_pool.tile([128, 512], F32, tag="b_sb", bufs=2,
                                          name="b_sb")
                    nc.gpsimd.partition_broadcast(b_sb[:], d_sb[:])
                    ytile = work_pool.tile([128, 512], BF16, tag="ytile",
                                           bufs=2, name="ytile")
                    nc.vector.tensor_tensor(
                        ytile[:], ps_y[:], b_sb[:], mybir.AluOpType.mult)
                    nc.sync.dma_start(
                        yt_piece[m][:, j * 512:(j + 1) * 512], ytile[:])

            # attention train: per head attention -> AllGather piece,
            # overlapped with the next head's projections. The gathered
            # pieces are loaded to SBUF on the gpsimd queue: pieces 0-2
            # between proj3 and attn3 (the x tiles they replace die when
            # proj3's matmuls drain), piece 3 after its AllGather, so
            # stage F can run during AG3 without queue head-of-line
            # blocking on the collective.
            ytfp_sb = [[None] * GROUPS for _ in range(HPC)]

            def load_piece(m):
                for p in range(GROUPS):
                    t = big_pool.tile([128, S], BF16, tag="big",
                                      name=f"ytf{m}_{p}")
                    nc.gpsimd.dma_start(
                        t[:], ytf_piece[m][p * 128:(p + 1) * 128, :])
                    ytfp_sb[m][p] = t

            for m in range(HPC):
                attn_head(m)
                nc.gpsimd.collective_compute(
                    "AllGather",
                    mybir.AluOpType.bypass,
                    replica_groups=[[0, 1, 2, 3], [4, 5, 6, 7]],
                    ins=[yt_piece[m].opt()],
                    outs=[ytf_piece[m].opt()],
                )
                if m + 1 < HPC:
                    proj_head(m + 1)
                    if m + 1 == HPC - 1:
                        for mm in range(HPC - 1):
                            load_piece(mm)
            load_piece(HPC - 1)

            # ---- stage F: output projection (512-col slice) ---------------
            # ytf_piece[m] rows: 4 ranks x head m -> global head 4*p + m

            for ms in range(NT):
                ps = psum_pool.tile([128, DQC], F32, tag="ps_big", bufs=2,
                                    name="ps_o")
                for m in range(HPC):
                    for p in range(GROUPS):
                        nc.tensor.matmul(
                            ps[:],
                            ytfp_sb[m][p][:, ms * 128:(ms + 1) * 128],
                            wo_sb[4 * p + m][:],
                            start=(m == 0 and p == 0),
                            stop=(m == HPC - 1 and p == GROUPS - 1),
                        )
                o_sb = work_pool.tile([128, DQC], F32, tag="o_sb", bufs=2,
                                      name="o_sb")
                nc.scalar.copy(o_sb[:], ps[:])
                nc.sync.dma_start(out[ms * 128:(ms + 1) * 128, :], o_sb[:])

    nc.compile()
    return nc


_NC_CACHE = None


def kernel(x, wq, wk, wv, wo, mask, sin, cos):
    global LAST_RESULTS, _NC_CACHE
    bf16 = ml_dtypes.bfloat16

    xt = np.ascontiguousarray(x.transpose(0, 2, 1)).astype(bf16)     # [B, D, S]
    wq_b = wq.astype(bf16)
    wk_b = (wk * (H ** -0.5)).astype(bf16)   # fold k scaling into wk
    wv_b = wv.astype(bf16)
    wo_b = wo.astype(bf16)

    # transposed rope tables; sign-folded sin handles rotate_half:
    #   q'[0:64]   = q[0:64]*cos[0:64]   + q[64:128]*(-sin[0:64])
    #   q'[64:128] = q[64:128]*cos[64:]  + q[0:64]  *(+sin[64:])
    cosT = np.ascontiguousarray(cos.T).astype(bf16)                  # [H, S]
    sinT = np.ascontiguousarray(sin.T).astype(np.float32)
    sinTs = sinT.copy()
    sinTs[0:H // 2, :] *= -1.0
    sinTs = sinTs.astype(bf16)

    # multiplicative causal mask for the 128x128 diagonal blocks, in
    # [kv, q] orientation, derived from the additive mask input
    tri = (mask[:128, :128].T == 0.0).astype(bf16)

    if _NC_CACHE is None:
        _NC_CACHE = _build()
    nc = _NC_CACHE

    in_maps = []
    for c in range(N_CORES):
        b, g = c // GROUPS, c % GROUPS
        cols = slice(g * DQC, (g + 1) * DQC)
        in_maps.append({
            "xt": xt[b],
            "wq": np.ascontiguousarray(wq_b[:, cols]),
            "wk": np.ascontiguousarray(wk_b[:, cols]),
            "wv": np.ascontiguousarray(wv_b[:, cols]),
            "wo": np.ascontiguousarray(wo_b[:, cols]),
            "cosT": cosT,
            "sinTs": sinTs,
            "tri": tri,
        })

    res = run_bass_kernel_spmd(nc, in_maps, core_ids=list(range(N_CORES)))
    LAST_RESULTS = res

    output = np.empty((B, S, D), dtype=np.float32)
    for c in range(N_CORES):
        b, g = c // GROUPS, c % GROUPS
        output[b, :, g * DQC:(g + 1) * DQC] = res.results[c]["out"]
    return output


# revision 17
# speedup vs baseline: 1.1509x; 1.0469x over previous
"""Trainium2 Bass kernel for a dense causal-attention transformer block.

Computes: qkv projections + RoPE + causal softmax attention + output
projection, matching the reference jax implementation with
B=2, S=2048, D=2048, 16 heads x 128 head-dim, on 8 NeuronCores.

Sharding: data-parallel over batch (2 groups of 4 cores) x tensor-parallel
over heads (4 heads per core). Attention is fully head-local; the only
communication is an AllGather of the per-core attention outputs (bf16)
within each 4-core batch group before the wo matmul, of which each core
computes a 512-wide output-column slice.
"""
import os
import sys
import types

sys.path.insert(0, "/opt/trn_rl_repo")

import numpy as np


def _install_ntff_hook():
    """Recreate the missing antenv.axon_hooks module so trace=True works."""
    try:
        import antenv

        if "antenv.axon_hooks" in sys.modules:
            return
        m = types.ModuleType("antenv.axon_hooks")
        m._hook = None

        def set_axon_ntff_profile_hook(h):
            m._hook = h

        def get_axon_ntff_profile_hook():
            return m._hook

        m.set_axon_ntff_profile_hook = set_axon_ntff_profile_hook
        m.get_axon_ntff_profile_hook = get_axon_ntff_profile_hook
        sys.modules["antenv.axon_hooks"] = m
        antenv.axon_hooks = m
        from trn_agent_boot.trn_boot import _ntff_profile_via_ctypes

        so = "/opt/axon/libaxon_pjrt.so"
        if os.path.exists(so):
            set_axon_ntff_profile_hook(_ntff_profile_via_ctypes(so))
    except Exception:
        pass


_install_ntff_hook()

import ml_dtypes
import concourse.bass as bass
import concourse.tile as tile
from concourse import bacc, mybir
from concourse.bass_utils import run_bass_kernel_spmd

BF16 = mybir.dt.bfloat16
F32 = mybir.dt.float32

B, S, D = 2, 2048, 2048
QH, H = 16, 128          # heads, head dim
N_CORES = 8
GROUPS = 4               # tensor-parallel groups per batch
HPC = QH // GROUPS       # heads per core = 4
DQC = HPC * H            # per-core projection width = 512
NT = S // 128            # 16 s/d tiles of 128
NC = S // 512            # 4 chunks of 512
KT = D // 128            # 16 contraction tiles

LAST_RESULTS = None      # test harness reads exec_time_ns from here


def _build():
    nc = bacc.Bacc("TRN2", target_bir_lowering=False, debug=False,
                   num_devices=N_CORES)

    xt = nc.dram_tensor("xt", [D, S], BF16, kind="ExternalInput")
    wq = nc.dram_tensor("wq", [D, DQC], BF16, kind="ExternalInput")
    wk = nc.dram_tensor("wk", [D, DQC], BF16, kind="ExternalInput")
    wv = nc.dram_tensor("wv", [D, DQC], BF16, kind="ExternalInput")
    wo = nc.dram_tensor("wo", [D, DQC], BF16, kind="ExternalInput")
    cosT = nc.dram_tensor("cosT", [H, S], BF16, kind="ExternalInput")
    sinTs = nc.dram_tensor("sinTs", [H, S], BF16, kind="ExternalInput")
    tri = nc.dram_tensor("tri", [128, 128], BF16, kind="ExternalInput")
    out = nc.dram_tensor("out", [S, DQC], F32, kind="ExternalOutput")

    with tile.TileContext(nc) as tc:
        with (
            tc.tile_pool(name="big", bufs=16) as big_pool,        # xt / ytf tiles
            tc.tile_pool(name="wts", bufs=16) as wts_pool,        # weight tiles
            tc.tile_pool(name="qkv", bufs=1) as qkv_pool,         # qt/kt/v/yt
            tc.tile_pool(name="small", bufs=1) as small_pool,     # constants
            tc.tile_pool(name="work", bufs=3) as work_pool,       # rope/at/tmp
            tc.tile_pool(name="psum", bufs=2, space="PSUM") as psum_pool,
            tc.tile_pool(name="dram", bufs=1, space="DRAM") as dram_pool,
        ):
            # ---- constants / inputs to SBUF -------------------------------
            cos_sb = small_pool.tile([H, S], BF16, tag="cos")
            nc.sync.dma_start(cos_sb[:], cosT[:])
            sin_sb = small_pool.tile([H, S], BF16, tag="sin")
            nc.sync.dma_start(sin_sb[:], sinTs[:])
            tri_sb = small_pool.tile([128, 128], BF16, tag="tri")
            nc.sync.dma_start(tri_sb[:], tri[:])
            ones_sb = small_pool.tile([128, 1], BF16, tag="ones")
            nc.vector.memset(ones_sb[:], 1.0)

            xt_sb = []
            wq_sb, wk_sb = [], []
            for kd in range(KT):
                r = slice(kd * 128, (kd + 1) * 128)
                t = big_pool.tile([128, S], BF16, tag="big", name=f"xt{kd}")
                nc.sync.dma_start(t[:], xt[r, :])
                xt_sb.append(t)
                for (lst, src, nm, tg) in ((wq_sb, wq, "wq", "w0"),
                                           (wk_sb, wk, "wk", "w1")):
                    w = wts_pool.tile([128, DQC], BF16, tag=tg,
                                      name=f"{nm}{kd}")
                    nc.sync.dma_start(w[:], src[r, :])
                    lst.append(w)

            # ---- pipelined stages ----------------------------------------
            # Per head-block m: project Q/K (head 0 first), attention, then
            # fire that head's AllGather piece while later heads project and
            # attend; stage F consumes gathered pieces as they arrive.
            qt_sb = [qkv_pool.tile([H, S], BF16, tag=f"qt{m}", name=f"qt{m}")
                     for m in range(HPC)]
            kt_sb = [qkv_pool.tile([H, S], BF16, tag=f"kt{m}", name=f"kt{m}")
                     for m in range(HPC)]
            v_sb = [qkv_pool.tile([128, DQC], BF16, tag=f"v{i}", name=f"v{i}")
                    for i in range(NT)]

            def proj_head(m):
                """Q^T and K^T projections + RoPE for head m."""
                for sc in range(NC):
                    scol = slice(sc * 512, (sc + 1) * 512)
                    for (w_list, dst) in ((wq_sb, qt_sb[m]),
                                          (wk_sb, kt_sb[m])):
                        ps = psum_pool.tile([128, 512], F32, tag="ps_big",
                                            bufs=2, name="ps")
                        for kd in range(KT):
                            nc.tensor.matmul(
                                ps[:],
                                w_list[kd][:, m * 128:(m + 1) * 128],
                                xt_sb[kd][:, scol],
                                start=(kd == 0), stop=(kd == KT - 1),
                            )
                        # RoPE: out = p*cos + swap_halves(p)*sin_signed
                        t_rot = work_pool.tile([128, 512], F32, tag="t_rot",
                                               bufs=2, name="t_rot")
                        nc.vector.tensor_copy(t_rot[0:64, :], ps[64:128, :])
                        nc.vector.tensor_copy(t_rot[64:128, :], ps[0:64, :])
                        t_cos = work_pool.tile([128, 512], F32, tag="t_cos",
                                               bufs=2, name="t_cos")
                        nc.vector.tensor_tensor(
                            t_cos[:], ps[:], cos_sb[:, scol],
                            mybir.AluOpType.mult)
                        t_sin = work_pool.tile([128, 512], F32, tag="t_sin",
                                               bufs=2, name="t_sin")
                        nc.vector.tensor_tensor(
                            t_sin[:], t_rot[:], sin_sb[:, scol],
                            mybir.AluOpType.mult)
                        nc.vector.tensor_tensor(
                            dst[:, scol], t_cos[:], t_sin[:],
                            mybir.AluOpType.add)

            proj_head(0)

            # wv gets its own slots (wq/wk stay live for later heads);
            # wo reuses them after the V projection drains
            wv_sb = []
            for kd in range(KT):
                w = wts_pool.tile([128, DQC], BF16, tag="w2", name=f"wv{kd}")
                nc.sync.dma_start(w[:], wv[kd * 128:(kd + 1) * 128, :])
                wv_sb.append(w)

            for i in range(NT):
                ps = psum_pool.tile([128, DQC], F32, tag="ps_big", bufs=2,
                                    name="ps_v")
                for kd in range(KT):
                    nc.tensor.matmul(
                        ps[:],
                        xt_sb[kd][:, i * 128:(i + 1) * 128],
                        wv_sb[kd][:],
                        start=(kd == 0), stop=(kd == KT - 1),
                    )
                nc.scalar.copy(v_sb[i][:], ps[:])

            wo_sb = []
            for kd in range(KT):
                w = wts_pool.tile([128, DQC], BF16, tag="w2", name=f"wo{kd}")
                nc.sync.dma_start(w[:], wo[kd * 128:(kd + 1) * 128, :])
                wo_sb.append(w)

            yt_piece = [dram_pool.tile([128, S], BF16, name=f"yt_p{m}")
                        for m in range(HPC)]
            ytf_piece = [dram_pool.tile([DQC, S], BF16, name=f"ytf_p{m}")
                         for m in range(HPC)]
            # head 3 gathers per q-chunk so its AllGather pipelines behind
            # the attention loop instead of serializing after it
            yt3_sub = [dram_pool.tile([128, 512], BF16, name=f"yt3_s{j}")
                       for j in range(NC)]
            ytf3_sub = [dram_pool.tile([DQC, 512], BF16, name=f"ytf3_s{j}")
                        for j in range(NC)]

            def attn_head(m):
                """Causal attention for head m -> yt_piece[m]."""
                for j in range(NC):
                    n_kv = 4 * j + 4
                    ps_y = psum_pool.tile([128, 512], F32, tag="ps_y",
                                          name="ps_y")
                    ps_d = psum_pool.tile([1, 512], F32, tag="ps_d",
                                          name="ps_d")
                    for t in range(n_kv):
                        c0 = max(0, (t - 4 * j) * 128)
                        ps_s = psum_pool.tile([128, 512], F32, tag="ps_s",
                                              name="ps_s")
                        nc.tensor.matmul(
                            ps_s[:, c0:512],
                            kt_sb[m][:, t * 128:(t + 1) * 128],
                            qt_sb[m][:, j * 512 + c0:(j + 1) * 512],
                            start=True, stop=True,
                        )
                        at = work_pool.tile([128, 512], BF16, tag="at",
                                            bufs=3, name="at")
                        nc.scalar.activation(
                            at[:, c0:512], ps_s[:, c0:512],
                            mybir.ActivationFunctionType.Exp)
                        if t >= 4 * j:
                            nc.vector.tensor_tensor(
                                at[:, c0:c0 + 128], at[:, c0:c0 + 128],
                                tri_sb[:], mybir.AluOpType.mult)
                        nc.tensor.matmul(
                            ps_y[:, c0:512],
                            v_sb[t][:, m * 128:(m + 1) * 128],
                            at[:, c0:512],
                            start=(t == 0), stop=(t == n_kv - 1),
                        )
                        nc.tensor.matmul(
                            ps_d[:, c0:512],
                            ones_sb[:],
                            at[:, c0:512],
                            start=(t == 0), stop=(t == n_kv - 1),
                        )
                    d_sb = work_pool.tile([1, 512], F32, tag="d_sb", bufs=2,
                                          name="d_sb")
                    nc.vector.reciprocal(d_sb[:], ps_d[:])
                    b_sb = work_pool.tile([128, 512], F32, tag="b_sb", bufs=2,
                                          name="b_sb")
                    nc.gpsimd.partition_broadcast(b_sb[:], d_sb[:])
                    ytile = work_pool.tile([128, 512], BF16, tag="ytile",
                                           bufs=2, name="ytile")
                    nc.vector.tensor_tensor(
                        ytile[:], ps_y[:], b_sb[:], mybir.AluOpType.mult)
                    if m == HPC - 1:
                        nc.sync.dma_start(yt3_sub[j][:], ytile[:])
                        nc.gpsimd.collective_compute(
                            "AllGather",
                            mybir.AluOpType.bypass,
                            replica_groups=[[0, 1, 2, 3], [4, 5, 6, 7]],
                            ins=[yt3_sub[j].opt()],
                            outs=[ytf3_sub[j].opt()],
                        )
                        if j < HPC - 1:
                            load_piece(j)
                    else:
                        nc.sync.dma_start(
                            yt_piece[m][:, j * 512:(j + 1) * 512], ytile[:])

            # attention train: per head attention -> AllGather piece,
            # overlapped with the next head's projections. The gathered
            # pieces are loaded to SBUF on the gpsimd queue: pieces 0-2
            # between proj3 and attn3 (the x tiles they replace die when
            # proj3's matmuls drain), piece 3 after its AllGather, so
            # stage F can run during AG3 without queue head-of-line
            # blocking on the collective.
            ytfp_sb = [[None] * GROUPS for _ in range(HPC)]
            # ytf3sub_sb[j][p]: head-3 sub-piece j, rank p (in dead v slots)
            ytf3sub_sb = [[None] * GROUPS for _ in range(NC)]

            def load_piece(m):
                for p in range(GROUPS):
                    t = big_pool.tile([128, S], BF16, tag="big",
                                      name=f"ytf{m}_{p}")
                    nc.sync.dma_start(
                        t[:], ytf_piece[m][p * 128:(p + 1) * 128, :])
                    ytfp_sb[m][p] = t

            for m in range(HPC):
                attn_head(m)
                if m < HPC - 1:
                    nc.gpsimd.collective_compute(
                        "AllGather",
                        mybir.AluOpType.bypass,
                        replica_groups=[[0, 1, 2, 3], [4, 5, 6, 7]],
                        ins=[yt_piece[m].opt()],
                        outs=[ytf_piece[m].opt()],
                    )
                    proj_head(m + 1)

            for j in range(NC):
                for p in range(GROUPS):
                    t = qkv_pool.tile([128, 512], BF16, tag=f"v{4 * j + p}",
                                      name=f"y3s{j}_{p}")
                    nc.gpsimd.dma_start(
                        t[:], ytf3_sub[j][p * 128:(p + 1) * 128, :])
                    ytf3sub_sb[j][p] = t

            # ---- stage F: output projection (512-col slice) ---------------
            # ytf_piece[m] rows: 4 ranks x head m -> global head 4*p + m

            for ms in range(NT):
                ps = psum_pool.tile([128, DQC], F32, tag="ps_big", bufs=2,
                                    name="ps_o")
                for m in range(HPC):
                    for p in range(GROUPS):
                        if m == HPC - 1:
                            lhsT = ytf3sub_sb[ms // 4][p][
                                :, (ms % 4) * 128:(ms % 4 + 1) * 128]
                        else:
                            lhsT = ytfp_sb[m][p][:, ms * 128:(ms + 1) * 128]
                        nc.tensor.matmul(
                            ps[:],
                            lhsT,
                            wo_sb[4 * p + m][:],
                            start=(m == 0 and p == 0),
                            stop=(m == HPC - 1 and p == GROUPS - 1),
                        )
                o_sb = work_pool.tile([128, DQC], F32, tag="o_sb", bufs=2,
                                      name="o_sb")
                nc.scalar.copy(o_sb[:], ps[:])
                nc.sync.dma_start(out[ms * 128:(ms + 1) * 128, :], o_sb[:])

    nc.compile()
    return nc


_NC_CACHE = None


def kernel(x, wq, wk, wv, wo, mask, sin, cos):
    global LAST_RESULTS, _NC_CACHE
    bf16 = ml_dtypes.bfloat16

    xt = np.ascontiguousarray(x.transpose(0, 2, 1)).astype(bf16)     # [B, D, S]
    wq_b = wq.astype(bf16)
    wk_b = (wk * (H ** -0.5)).astype(bf16)   # fold k scaling into wk
    wv_b = wv.astype(bf16)
    wo_b = wo.astype(bf16)

    # transposed rope tables; sign-folded sin handles rotate_half:
    #   q'[0:64]   = q[0:64]*cos[0:64]   + q[64:128]*(-sin[0:64])
    #   q'[64:128] = q[64:128]*cos[64:]  + q[0:64]  *(+sin[64:])
    cosT = np.ascontiguousarray(cos.T).astype(bf16)                  # [H, S]
    sinT = np.ascontiguousarray(sin.T).astype(np.float32)
    sinTs = sinT.copy()
    sinTs[0:H // 2, :] *= -1.0
    sinTs = sinTs.astype(bf16)

    # multiplicative causal mask for the 128x128 diagonal blocks, in
    # [kv, q] orientation, derived from the additive mask input
    tri = (mask[:128, :128].T == 0.0).astype(bf16)

    if _NC_CACHE is None:
        _NC_CACHE = _build()
    nc = _NC_CACHE

    in_maps = []
    for c in range(N_CORES):
        b, g = c // GROUPS, c % GROUPS
        cols = slice(g * DQC, (g + 1) * DQC)
        in_maps.append({
            "xt": xt[b],
            "wq": np.ascontiguousarray(wq_b[:, cols]),
            "wk": np.ascontiguousarray(wk_b[:, cols]),
            "wv": np.ascontiguousarray(wv_b[:, cols]),
            "wo": np.ascontiguousarray(wo_b[:, cols]),
            "cosT": cosT,
            "sinTs": sinTs,
            "tri": tri,
        })

    res = run_bass_kernel_spmd(nc, in_maps, core_ids=list(range(N_CORES)))
    LAST_RESULTS = res

    output = np.empty((B, S, D), dtype=np.float32)
    for c in range(N_CORES):
        b, g = c // GROUPS, c % GROUPS
        output[b, :, g * DQC:(g + 1) * DQC] = res.results[c]["out"]
    return output
